# revision 2
# baseline (speedup 1.0000x reference)
"""Trainium2 Bass kernel for ExpertMLP: out = relu(x @ W_fc.T)^2 @ W_proj.T.

Sharding: 4-way tokens x 2-way hidden across 8 NeuronCores.
Each core computes a partial out^T[:, t_shard] contracted over its hidden
half; the host sums the two hidden halves (fp16 partials, fp32 accumulate)
and transposes while unsharding.

Per-core kernel (T_S=2048 tokens, HID_S=2048 hidden, DIM=1024), fp16
matmul operands with fp32 PSUM accumulation. The token shard is split into
two 1024-token halves and the phases are pipelined

    mm1(h0) -> mm1(h1) -> mm2(h0) -> mm2(h1)

so there is no global mm1->mm2 barrier (mm2(h0) only needs h0, which
drained while mm1(h1) streamed) and consecutive reps overlap at the
boundary. Within a group the contraction loop is innermost (t-chunk outer),
so each 512-token PSUM tile accumulates in consecutive matmuls and its
activation (ScalarE relu, VectorE square -> h fp16) overlaps the next
chunk's matmuls. mm1 and mm2 use disjoint PSUM banks (tags 0/1 vs 2/3 in
two pools) because both phases are in flight at once.

x, h and both weights stay SBUF-resident; per rep only out^T (fp16) leaves
the core. Startup orders DMAs x[t0], wfc[j0..j1], x[t1], ... so the first
real matmul ungates after ~3.6us of DMA; a short garbage-operand prewarm
keeps the PE busy (HAM warm) until then.
"""

import numpy as np

import concourse.mybir as mybir
import concourse.tile as tile
from concourse import bacc
from concourse import bass_utils

T, DIM, HID = 8192, 1024, 4096
N_CORES = 8
TOK_WAYS, HID_WAYS = 4, 2
T_S = T // TOK_WAYS        # 2048 tokens per core
HID_S = HID // HID_WAYS    # 2048 hidden units per core
P = 128
F32 = mybir.dt.float32
F16 = mybir.dt.float16

T_CHUNK = 512
KD = DIM // P              # 8 contraction chunks for mm1
JC = HID_S // P            # 16 j-chunks (also mm2 contraction chunks)
DC = DIM // P              # 8 output-dim chunks for mm2
N_WARM = 12


def build_nc(reps: int = 1):
    t_chunk = T_CHUNK
    nc = bacc.Bacc("TRN2", target_bir_lowering=False, debug=False)
    xT = nc.dram_tensor("xT", [DIM, T_S], F16, kind="ExternalInput")
    wfcT = nc.dram_tensor("wfcT", [DIM, HID_S], F16, kind="ExternalInput")
    wprojT = nc.dram_tensor("wprojT", [HID_S, DIM], F16, kind="ExternalInput")
    outT = nc.dram_tensor("outT", [DIM, T_S], F16, kind="ExternalOutput")

    xT_r = xT.ap().rearrange("(o p) t -> p o t", p=P)
    wfcT_r = wfcT.ap().rearrange("(o p) h -> p o h", p=P)
    wprojT_r = wprojT.ap().rearrange("(o p) d -> p o d", p=P)
    outT_r = outT.ap().rearrange("(o p) t -> p o t", p=P)

    with tile.TileContext(nc) as tc:
        with (
            tc.tile_pool(name="weights", bufs=1) as wpool,
            tc.tile_pool(name="xin", bufs=1) as xpool,
            tc.tile_pool(name="hact", bufs=1) as hpool,
            tc.tile_pool(name="tmp", bufs=3) as tpool,
            tc.tile_pool(name="outp", bufs=3) as opool,
            tc.tile_pool(name="ps_a", bufs=1, space="PSUM") as ps_a,
            tc.tile_pool(name="ps_b", bufs=1, space="PSUM") as ps_b,
        ):
            pools = [ps_a, ps_b]
            wfc_sb = wpool.tile([P, KD, HID_S], F16)
            wproj_sb = wpool.tile([P, JC, DIM], F16)

            # PE prewarm: matmuls on a scratch tile (only the stationary
            # columns are zeroed; garbage elsewhere is fine since every
            # PSUM bank is later reset by a start=True group) bridge the
            # initial DMA wait and take the HAM clock gate to 2.4GHz.
            warm_sb = wpool.tile([P, t_chunk], F16)
            nc.gpsimd.memset(warm_sb[:, :P], 0.0)
            ps_w = ps_a.tile([P, t_chunk], F32, tag="a0")
            for _ in range(N_WARM):
                nc.tensor.matmul(ps_w[:], lhsT=warm_sb[:, :P], rhs=warm_sb[:],
                                 start=True, stop=True)

            # DMA order: first x chunk + first wfc columns first so mm1 j=0
            # ungates early; wproj (needed ~100us in) last.
            x_sb = xpool.tile([P, KD, T_S], F16)
            H_SPLIT = 256
            nc.sync.dma_start(x_sb[:, :, 0:t_chunk], xT_r[:, :, 0:t_chunk])
            nc.sync.dma_start(wfc_sb[:, :, 0:H_SPLIT], wfcT_r[:, :, 0:H_SPLIT])
            for t in range(1, 4):
                tsl = slice(t * t_chunk, (t + 1) * t_chunk)
                nc.sync.dma_start(x_sb[:, :, tsl], xT_r[:, :, tsl])
                if t < 3:
                    sl = slice(t * H_SPLIT, (t + 1) * H_SPLIT)
                    nc.sync.dma_start(wfc_sb[:, :, sl], wfcT_r[:, :, sl])
            for js in range(3, HID_S // H_SPLIT):
                sl = slice(js * H_SPLIT, (js + 1) * H_SPLIT)
                nc.sync.dma_start(wfc_sb[:, :, sl], wfcT_r[:, :, sl])
            for js in range(4):
                sl = slice(js * (JC // 4), (js + 1) * (JC // 4))
                nc.sync.dma_start(wproj_sb[:, sl, :], wprojT_r[:, sl, :])

            def mm1_group(j, h_sb, tlist):
                pool = pools[j % 2]
                pfx = "a" if j % 2 == 0 else "b"
                wsl = wfc_sb[:, :, j * P:(j + 1) * P]
                for i, t in enumerate(tlist):
                    ps_t = pool.tile([P, t_chunk], F32,
                                     tag=f"{pfx}{i}", name=f"{pfx}{i}")
                    for k in range(KD):
                        nc.tensor.matmul(
                            ps_t[:], lhsT=wsl[:, k, :],
                            rhs=x_sb[:, k, t * t_chunk:(t + 1) * t_chunk],
                            start=(k == 0), stop=(k == KD - 1),
                        )
                    relu_t = tpool.tile([P, t_chunk], F16, tag="relu")
                    nc.scalar.activation(relu_t[:], ps_t[:],
                                         mybir.ActivationFunctionType.Relu)
                    lo = (t - tlist[0]) * t_chunk
                    nc.vector.tensor_mul(out=h_sb[:, j, lo:lo + t_chunk],
                                         in0=relu_t[:], in1=relu_t[:])

            def mm2_group(dc, h_sb, tlist):
                pool = pools[dc % 2]
                pfx = "a" if dc % 2 == 0 else "b"
                wsl = wproj_sb[:, :, dc * P:(dc + 1) * P]
                for i, t in enumerate(tlist):
                    po_t = pool.tile([P, t_chunk], F32,
                                     tag=f"{pfx}{2 + i}", name=f"{pfx}{2 + i}")
                    lo = (t - tlist[0]) * t_chunk
                    for j in range(JC):
                        nc.tensor.matmul(
                            po_t[:], lhsT=wsl[:, j, :],
                            rhs=h_sb[:, j, lo:lo + t_chunk],
                            start=(j == 0), stop=(j == JC - 1),
                        )
                    o_sb = opool.tile([P, t_chunk], F16, tag="o")
                    nc.vector.tensor_copy(out=o_sb[:], in_=po_t[:])
                    nc.sync.dma_start(
                        outT_r[:, dc, t * t_chunk:(t + 1) * t_chunk], o_sb[:])

            def body(_iv=None):
                h0 = hpool.tile([P, JC, T_S // 2], F16, tag="h0")
                h1 = hpool.tile([P, JC, T_S // 2], F16, tag="h1")
                for j in range(JC):
                    mm1_group(j, h0, [0, 1])
                for j in range(JC):
                    mm1_group(j, h1, [2, 3])
                for dc in range(DC):
                    mm2_group(dc, h0, [0, 1])
                for dc in range(DC):
                    mm2_group(dc, h1, [2, 3])

            body()
            if reps > 1:
                if reps <= 4:
                    for _ in range(reps - 1):
                        body()
                else:
                    with tc.For_i(0, reps - 1, 1) as iv:
                        body(iv)

    nc.compile()
    return nc


_NC_CACHE = {}


def _get_nc(reps: int = 1):
    if reps not in _NC_CACHE:
        _NC_CACHE[reps] = build_nc(reps)
    return _NC_CACHE[reps]


def make_in_maps(x, W_fc, W_proj):
    xT = np.ascontiguousarray(x.T.astype(np.float16))  # [DIM, T]
    wfcT16 = {}
    wprojT16 = {}
    for hid in range(HID_WAYS):
        hsl = slice(hid * HID_S, (hid + 1) * HID_S)
        wfcT16[hid] = np.ascontiguousarray(W_fc[hsl, :].T.astype(np.float16))
        wprojT16[hid] = np.ascontiguousarray(W_proj[:, hsl].T.astype(np.float16))
    in_maps = []
    for c in range(N_CORES):
        tok, hid = c // HID_WAYS, c % HID_WAYS
        in_maps.append({
            "xT": np.ascontiguousarray(xT[:, tok * T_S:(tok + 1) * T_S]),
            "wfcT": wfcT16[hid],
            "wprojT": wprojT16[hid],
        })
    return in_maps


def assemble_out(results):
    out = np.empty((T, DIM), dtype=np.float32)
    for tok in range(TOK_WAYS):
        acc = results[tok * HID_WAYS]["outT"].astype(np.float32)
        for hid in range(1, HID_WAYS):
            acc += results[tok * HID_WAYS + hid]["outT"].astype(np.float32)
        out[tok * T_S:(tok + 1) * T_S] = acc.T
    return out


def kernel(x, W_fc, W_proj):
    assert x.shape == (T, DIM) and W_fc.shape == (HID, DIM) and W_proj.shape == (DIM, HID)
    nc = _get_nc(reps=1)
    in_maps = make_in_maps(
        np.asarray(x, np.float32),
        np.asarray(W_fc, np.float32),
        np.asarray(W_proj, np.float32),
    )
    res = bass_utils.run_bass_kernel_spmd(nc, in_maps, core_ids=list(range(N_CORES)))
    return assemble_out(res.results)


# revision 3
# speedup vs baseline: 1.0762x; 1.0762x over previous
"""Trainium2 Bass kernel for ExpertMLP: out = relu(x @ W_fc.T)^2 @ W_proj.T.

Sharding: 4-way tokens x 2-way hidden across 8 NeuronCores.
Each core computes a partial out^T[:, t_shard] contracted over its hidden
half; the host sums the two hidden halves (fp16 partials, fp32 accumulate)
and transposes while unsharding.

Per-core kernel (T_S=2048 tokens, HID_S=2048 hidden, DIM=1024), fp16
matmul operands with fp32 PSUM accumulation. The token shard is split into
two 1024-token halves and the phases are pipelined

    mm1(h0) -> mm1(h1) -> mm2(h0) -> mm2(h1)

so there is no global mm1->mm2 barrier (mm2(h0) only needs h0, which
drained while mm1(h1) streamed) and consecutive reps overlap at the
boundary. Within a group the contraction loop is innermost (t-chunk outer),
so each 512-token PSUM tile accumulates in consecutive matmuls and its
activation (ScalarE relu, VectorE square -> h fp16) overlaps the next
chunk's matmuls. mm1 and mm2 use disjoint PSUM banks (tags 0/1 vs 2/3 in
two pools) because both phases are in flight at once.

x, h and both weights stay SBUF-resident; per rep only out^T (fp16) leaves
the core. Startup orders DMAs x[t0], wfc[j0..j1], x[t1], ... so the first
real matmul ungates after ~3.6us of DMA; a short garbage-operand prewarm
keeps the PE busy (HAM warm) until then.
"""

import numpy as np

import concourse.mybir as mybir
import concourse.tile as tile
from concourse import bacc
from concourse import bass_utils

T, DIM, HID = 8192, 1024, 4096
N_CORES = 8
TOK_WAYS, HID_WAYS = 4, 2
T_S = T // TOK_WAYS        # 2048 tokens per core
HID_S = HID // HID_WAYS    # 2048 hidden units per core
P = 128
F32 = mybir.dt.float32
F16 = mybir.dt.float16

T_CHUNK = 512
KD = DIM // P              # 8 contraction chunks for mm1
JC = HID_S // P            # 16 j-chunks (also mm2 contraction chunks)
DC = DIM // P              # 8 output-dim chunks for mm2
N_WARM = 12


def build_nc(reps: int = 1):
    t_chunk = T_CHUNK
    nc = bacc.Bacc("TRN2", target_bir_lowering=False, debug=False)
    xT = nc.dram_tensor("xT", [DIM, T_S], F16, kind="ExternalInput")
    wfcT = nc.dram_tensor("wfcT", [DIM, HID_S], F16, kind="ExternalInput")
    wprojT = nc.dram_tensor("wprojT", [HID_S, DIM], F16, kind="ExternalInput")
    outT = nc.dram_tensor("outT", [DIM, T_S], F16, kind="ExternalOutput")

    xT_r = xT.ap().rearrange("(o p) t -> p o t", p=P)
    wfcT_r = wfcT.ap().rearrange("(o p) h -> p o h", p=P)
    wprojT_r = wprojT.ap().rearrange("(o p) d -> p o d", p=P)
    outT_r = outT.ap().rearrange("(o p) t -> p o t", p=P)

    with tile.TileContext(nc) as tc:
        with (
            tc.tile_pool(name="weights", bufs=1) as wpool,
            tc.tile_pool(name="xin", bufs=1) as xpool,
            tc.tile_pool(name="hact", bufs=1) as hpool,
            tc.tile_pool(name="tmp", bufs=3) as tpool,
            tc.tile_pool(name="outp", bufs=3) as opool,
            tc.tile_pool(name="ps_a", bufs=1, space="PSUM") as ps_a,
            tc.tile_pool(name="ps_b", bufs=1, space="PSUM") as ps_b,
        ):
            pools = [ps_a, ps_b]
            wfc_sb = wpool.tile([P, KD, HID_S], F16)
            wproj_sb = wpool.tile([P, JC, DIM], F16)

            # PE prewarm: matmuls on a scratch tile (only the stationary
            # columns are zeroed; garbage elsewhere is fine since every
            # PSUM bank is later reset by a start=True group) bridge the
            # initial DMA wait and take the HAM clock gate to 2.4GHz.
            warm_sb = wpool.tile([P, t_chunk], F16)
            nc.gpsimd.memset(warm_sb[:, :P], 0.0)
            ps_w = ps_a.tile([P, t_chunk], F32, tag="a0")
            for _ in range(N_WARM):
                nc.tensor.matmul(ps_w[:], lhsT=warm_sb[:, :P], rhs=warm_sb[:],
                                 start=True, stop=True)

            # DMA order: first x chunk + first wfc columns first so mm1 j=0
            # ungates early; wproj (needed ~100us in) last.
            x_sb = xpool.tile([P, KD, T_S], F16)
            H_SPLIT = 256
            nc.sync.dma_start(x_sb[:, :, 0:t_chunk], xT_r[:, :, 0:t_chunk])
            nc.sync.dma_start(wfc_sb[:, :, 0:H_SPLIT], wfcT_r[:, :, 0:H_SPLIT])
            for t in range(1, 4):
                tsl = slice(t * t_chunk, (t + 1) * t_chunk)
                nc.sync.dma_start(x_sb[:, :, tsl], xT_r[:, :, tsl])
                if t < 3:
                    sl = slice(t * H_SPLIT, (t + 1) * H_SPLIT)
                    nc.sync.dma_start(wfc_sb[:, :, sl], wfcT_r[:, :, sl])
            for js in range(3, HID_S // H_SPLIT):
                sl = slice(js * H_SPLIT, (js + 1) * H_SPLIT)
                nc.sync.dma_start(wfc_sb[:, :, sl], wfcT_r[:, :, sl])
            for js in range(4):
                sl = slice(js * (JC // 4), (js + 1) * (JC // 4))
                nc.sync.dma_start(wproj_sb[:, sl, :], wprojT_r[:, sl, :])

            def mm1_group(j, h_sb, tlist):
                pool = pools[j % 2]
                pfx = "a" if j % 2 == 0 else "b"
                wsl = wfc_sb[:, :, j * P:(j + 1) * P]
                for i, t in enumerate(tlist):
                    ps_t = pool.tile([P, t_chunk], F32,
                                     tag=f"{pfx}{i}", name=f"{pfx}{i}")
                    for k in range(KD):
                        nc.tensor.matmul(
                            ps_t[:], lhsT=wsl[:, k, :],
                            rhs=x_sb[:, k, t * t_chunk:(t + 1) * t_chunk],
                            start=(k == 0), stop=(k == KD - 1),
                        )
                    relu_t = tpool.tile([P, t_chunk], F16, tag="relu")
                    nc.scalar.activation(relu_t[:], ps_t[:],
                                         mybir.ActivationFunctionType.Relu)
                    lo = (t - tlist[0]) * t_chunk
                    nc.vector.tensor_mul(out=h_sb[:, j, lo:lo + t_chunk],
                                         in0=relu_t[:], in1=relu_t[:])

            def mm2_group(dc, h_sb, tlist):
                pool = pools[dc % 2]
                pfx = "a" if dc % 2 == 0 else "b"
                wsl = wproj_sb[:, :, dc * P:(dc + 1) * P]
                for i, t in enumerate(tlist):
                    po_t = pool.tile([P, t_chunk], F32,
                                     tag=f"{pfx}{2 + i}", name=f"{pfx}{2 + i}")
                    lo = (t - tlist[0]) * t_chunk
                    for j in range(JC):
                        nc.tensor.matmul(
                            po_t[:], lhsT=wsl[:, j, :],
                            rhs=h_sb[:, j, lo:lo + t_chunk],
                            start=(j == 0), stop=(j == JC - 1),
                        )
                    o_sb = opool.tile([P, t_chunk], F16, tag="o")
                    nc.vector.tensor_copy(out=o_sb[:], in_=po_t[:])
                    nc.sync.dma_start(
                        outT_r[:, dc, t * t_chunk:(t + 1) * t_chunk], o_sb[:])

            def body(_iv=None):
                h0 = hpool.tile([P, JC, T_S // 2], F16, tag="h0")
                h1 = hpool.tile([P, JC, T_S // 2], F16, tag="h1")
                for j in range(JC):
                    mm1_group(j, h0, [0, 1])
                for j in range(JC):
                    mm1_group(j, h1, [2, 3])
                for dc in range(DC):
                    mm2_group(dc, h0, [0, 1])
                for dc in range(DC):
                    mm2_group(dc, h1, [2, 3])

            body()
            if reps > 1:
                if reps <= 4:
                    for _ in range(reps - 1):
                        body()
                else:
                    # Unroll x2 inside the hardware loop: halves the
                    # For_i loop-back resync cost (~1-3us/rep measured).
                    n = reps - 1
                    if n % 2 == 1:
                        body()
                        n -= 1
                    with tc.For_i(0, n // 2, 1) as iv:
                        body(iv)
                        body(iv)

    nc.compile()
    return nc


_NC_CACHE = {}


def _get_nc(reps: int = 1):
    if reps not in _NC_CACHE:
        _NC_CACHE[reps] = build_nc(reps)
    return _NC_CACHE[reps]


def make_in_maps(x, W_fc, W_proj):
    xT = np.ascontiguousarray(x.T.astype(np.float16))  # [DIM, T]
    wfcT16 = {}
    wprojT16 = {}
    for hid in range(HID_WAYS):
        hsl = slice(hid * HID_S, (hid + 1) * HID_S)
        wfcT16[hid] = np.ascontiguousarray(W_fc[hsl, :].T.astype(np.float16))
        wprojT16[hid] = np.ascontiguousarray(W_proj[:, hsl].T.astype(np.float16))
    in_maps = []
    for c in range(N_CORES):
        tok, hid = c // HID_WAYS, c % HID_WAYS
        in_maps.append({
            "xT": np.ascontiguousarray(xT[:, tok * T_S:(tok + 1) * T_S]),
            "wfcT": wfcT16[hid],
            "wprojT": wprojT16[hid],
        })
    return in_maps


def assemble_out(results):
    out = np.empty((T, DIM), dtype=np.float32)
    for tok in range(TOK_WAYS):
        acc = results[tok * HID_WAYS]["outT"].astype(np.float32)
        for hid in range(1, HID_WAYS):
            acc += results[tok * HID_WAYS + hid]["outT"].astype(np.float32)
        out[tok * T_S:(tok + 1) * T_S] = acc.T
    return out


def kernel(x, W_fc, W_proj):
    assert x.shape == (T, DIM) and W_fc.shape == (HID, DIM) and W_proj.shape == (DIM, HID)
    nc = _get_nc(reps=1)
    in_maps = make_in_maps(
        np.asarray(x, np.float32),
        np.asarray(W_fc, np.float32),
        np.asarray(W_proj, np.float32),
    )
    res = bass_utils.run_bass_kernel_spmd(nc, in_maps, core_ids=list(range(N_CORES)))
    return assemble_out(res.results)
